# revision 15
# baseline (speedup 1.0000x reference)
"""DampedLinOSSLayer Trainium2 kernel v5 (8 NeuronCores, batch-sharded).

Radix-4 time decimation on top of the gauge-factorized scan:
  x_t = lam x_{t-1} + B u_t  decimated by Q=4: z_k = lam^4 z_{k-1} + d_k with
  d_k = sum_j lam^{3-j} B u_{4k+j}.  The lam^{3-j} factors fold into 4 copies
  of the B weights, so the intra-group reduction runs on the PE (4
  accumulating matmuls over phase-deinterleaved input columns).  The DVE only
  rotates/scans the decimated (L/4) grid: gauge y_k = r^4 y_{k-1} + cf_k with
  cf = e^{-i4th k} d; one merged scan instruction covers all four
  (half, comp) segments via zeroed coefficient columns at segment starts.
  Output for t = 4k+j:
    out = Re(C lam^{j+1} z_{k-1}) + sum_{i<=j} M_{j-i} u_{4k+i}
  with M_d = Re(C lam^d B) + diag(D)[d=0].  The z-planes are combined on the
  DVE (z_re, z_im) so the C projection is 2 matmuls per (half, phase); the
  one-column shift of z rides the matmul output AP.  The u-term is a
  10-matmul causal phase convolution.  All rotation tables are plane-views
  of a single e^{+-i4th k} table.

Host side: input is pre-transposed/deinterleaved to [H, 4, 512] per batch
(plain contiguous DMA, no device transpose); output phases [4, H, 512] fp16
are re-interleaved on the host.
"""

import numpy as np

BATCH, LENGTH, HIDDEN, P = 32, 2048, 128, 256
N_CORES = 8
BPC = BATCH // N_CORES
Q = 4
K = LENGTH // Q

XIN_DTYPE = np.float16

_COMPILED = {}


def _build_program():
    import concourse.bacc as bacc
    import concourse.mybir as mybir
    from concourse.tile import TileContext

    f32 = mybir.dt.float32
    fp16 = mybir.dt.float16
    mm = mybir.AluOpType.mult
    ad = mybir.AluOpType.add

    nc = bacc.Bacc("TRN2", target_bir_lowering=False, debug=False,
                   num_devices=N_CORES)

    # host-deinterleaved input: xin[b, h, j, k] = u[b, 4k+j, h]
    xin = nc.dram_tensor("xin", [BPC, HIDDEN, Q, K], fp16,
                         kind="ExternalInput").ap()
    # B weights with lam^{3-j} folded: [h, half, comp, j, p]
    bw = nc.dram_tensor("bw", [HIDDEN, 2, 2, Q, 128], fp16,
                        kind="ExternalInput").ap()
    # per-phase C weights C*lam^{j+1}: [p, half, j, wt(0=re,1=-im), h]
    cw = nc.dram_tensor("cw", [128, 2, Q, 2, HIDDEN], fp16,
                        kind="ExternalInput").ap()
    # phase-convolution weights M_d^T: [h_in, d, h_out]
    m2w = nc.dram_tensor("m2w", [HIDDEN, Q, HIDDEN], fp16,
                         kind="ExternalInput").ap()
    # rotation tables, phase 4*th*k: [p, half, tab, plane, k]
    #   tab0 = [cos | -sin], tab1 = [sin | cos]
    epre = nc.dram_tensor("epre", [128, 2, 2, 2, K], fp16,
                          kind="ExternalInput").ap()
    # r^4 per (p, half) for the scan coefficient
    rcol = nc.dram_tensor("rcol", [128, 2], f32, kind="ExternalInput").ap()
    # output phases; host interleaves
    out = nc.dram_tensor("out", [BPC, Q, HIDDEN, K], fp16,
                         kind="ExternalOutput").ap()

    with TileContext(nc) as tc:
        with (
            tc.tile_pool(name="const", bufs=1) as cpool,
            tc.tile_pool(name="intp", bufs=3) as intr_pool,
            tc.tile_pool(name="dsb", bufs=2) as dsb_pool,
            tc.tile_pool(name="t12", bufs=2) as t12_pool,
            tc.tile_pool(name="cbuf", bufs=2) as cbuf_pool,
            tc.tile_pool(name="ybuf", bufs=2) as ybuf_pool,
            tc.tile_pool(name="xbuf", bufs=2) as xbuf_pool,
            tc.tile_pool(name="obuf", bufs=8) as obuf_pool,
            tc.tile_pool(name="psb", bufs=2, space="PSUM") as psb,
            tc.tile_pool(name="pso", bufs=4, space="PSUM") as pso,
        ):
            bw_t = cpool.tile([HIDDEN, 2, 2, Q, 128], fp16, tag="bw")
            cw_t = cpool.tile([128, 2, Q, 2, HIDDEN], fp16, tag="cw")
            m2w_t = cpool.tile([HIDDEN, Q, HIDDEN], fp16, tag="m2w")
            epre_t = cpool.tile([128, 2, 2, 2, K], fp16, tag="epre")
            rcol_t = cpool.tile([128, 2], f32, tag="rcol")
            # DMA order = need order: batch-0 B-proj wants bw+xin first, the
            # first pre-rotation wants epre+rcol, the C projection cw/m2w.
            inT = [None] * (BPC + 1)
            inT[0] = intr_pool.tile([HIDDEN, Q, K], fp16, tag="inT",
                                    name="inT0")
            nc.sync.dma_start(rcol_t[:], rcol[:])
            nc.sync.dma_start(bw_t[:, 0, 0], bw[:, 0, 0])
            nc.sync.dma_start(inT[0][:], xin[0])
            nc.sync.dma_start(bw_t[:, 0, 1], bw[:, 0, 1])
            nc.sync.dma_start(bw_t[:, 1], bw[:, 1])
            nc.sync.dma_start(epre_t[:, 0], epre[:, 0])
            nc.sync.dma_start(epre_t[:, 1], epre[:, 1])
            nc.sync.dma_start(m2w_t[:], m2w[:])
            nc.sync.dma_start(cw_t[:], cw[:])

            # scan coefficient [128, (half comp k)]: r^4, zero at segment
            # starts so one scan instruction covers 4 independent segments.
            rz_t = cpool.tile([128, 2, 2, K], f32, tag="rz")
            for half in range(2):
                nc.vector.memset(rz_t[:, half], 1.0)
                nc.vector.tensor_scalar_mul(
                    rz_t[:, half], rz_t[:, half],
                    rcol_t[:, half:half + 1])
            nc.vector.memset(rz_t[:, :, :, 0:1], 0.0)

            wtile = cpool.tile([128, 128], fp16, tag="warm")
            nc.gpsimd.memset(wtile[:], 0.0)
            wps = psb.tile([128, 2, K], f32, tag="d", name="warmps")
            for _ in range(24):
                nc.tensor.matmul(wps[:, 0, 0:128], wtile[:], wtile[:],
                                 start=True, stop=True)

            dsb = [None] * (BPC + 1)

            def bproj(b):
                # B-projection of batch b: 4 accumulating matmuls per
                # (half, comp) over the phase-deinterleaved input columns,
                # then PSUM->SBUF fp16 copies into one [p, half, comp, k]
                # tile.
                ds = dsb_pool.tile([128, 2, 2, K], fp16, tag="ds",
                                   name=f"ds{b}")
                for half in range(2):
                    d = psb.tile([128, 2, K], f32, tag="d")
                    for comp in range(2):
                        for j in range(Q):
                            nc.tensor.matmul(
                                d[:, comp], bw_t[:, half, comp, j],
                                inT[b][:, j], start=(j == 0), stop=(j == Q - 1))
                    nc.scalar.copy(ds[:, half], d[:])
                dsb[b] = ds

            bproj(0)
            for b in range(BPC):
                # prefetch + B-proj one batch ahead so the DVE chain of
                # batch b never waits on the PE/ScalarE front end.
                if b + 1 < BPC:
                    inT[b + 1] = intr_pool.tile([HIDDEN, Q, K], fp16,
                                                tag="inT",
                                                name=f"inT{b + 1}")
                    nc.sync.dma_start(inT[b + 1][:], xin[b + 1])
                    bproj(b + 1)

                # ---- per-half DVE chain; h0 scan/post overlap h1 ----
                cf = cbuf_pool.tile([128, 2, 2, K], fp16, tag="cf",
                                    name=f"cf{b}")
                yb = ybuf_pool.tile([128, 2, 2, K], fp16, tag="y",
                                    name=f"y{b}")
                zt = xbuf_pool.tile([128, 2, 2, K], fp16, tag="zt",
                                    name=f"zt{b}")

                def pre_rot(half):
                    # cf_re = d_re cos + d_im sin ; cf_im = -d_re sin + d_im cos
                    t12 = t12_pool.tile([128, 2, 2, K], fp16, tag="t12",
                                        name=f"t12_{b}_{half}")
                    nc.vector.tensor_mul(
                        t12[:, 0],
                        dsb[b][:, half, 0:1, :].to_broadcast([128, 2, K]),
                        epre_t[:, half, 0])
                    nc.vector.tensor_mul(
                        t12[:, 1],
                        dsb[b][:, half, 1:2, :].to_broadcast([128, 2, K]),
                        epre_t[:, half, 1])
                    nc.vector.tensor_add(cf[:, half], t12[:, 0], t12[:, 1])

                def pre_rot_merged():
                    # both halves in three FD-2048 ops (less fixed overhead)
                    t12 = t12_pool.tile([128, 2, 2, 2, K], fp16, tag="t12",
                                        name=f"t12m_{b}")
                    nc.vector.tensor_mul(
                        t12[:, 0],
                        dsb[b][:, :, 0:1, :].to_broadcast([128, 2, 2, K]),
                        epre_t[:, :, 0])
                    nc.vector.tensor_mul(
                        t12[:, 1],
                        dsb[b][:, :, 1:2, :].to_broadcast([128, 2, 2, K]),
                        epre_t[:, :, 1])
                    nc.vector.tensor_add(cf[:], t12[:, 0], t12[:, 1])

                def scan(half):
                    nc.vector.tensor_tensor_scan(
                        yb[:, half].rearrange("p c k -> p (c k)"),
                        rz_t[:, half].rearrange("p c k -> p (c k)"),
                        cf[:, half].rearrange("p c k -> p (c k)"), 0.0,
                        op0=mm, op1=ad)

                def post_rot(half):
                    # z_re = y_re cos - y_im sin ; z_im = y_re sin + y_im cos
                    # the sum rides a SWDGE SBUF->SBUF DMA (CCE add), keeping
                    # the DVE to the two products.
                    t34 = t12_pool.tile([128, 2, K], fp16, tag="t34",
                                        name=f"t34_{b}_{half}")
                    nc.vector.tensor_mul(
                        zt[:, half],
                        yb[:, half, 0:1, :].to_broadcast([128, 2, K]),
                        epre_t[:, half, :, 0, :])
                    nc.vector.tensor_mul(
                        t34[:],
                        yb[:, half, 1:2, :].to_broadcast([128, 2, K]),
                        epre_t[:, half, :, 1, :])
                    if b == BPC - 1:
                        # tail: the DVE add (~0.7us) beats the SWDGE CCE-add
                        # round trip (~2-3us) when nothing overlaps it.
                        nc.vector.tensor_add(zt[:, half], zt[:, half], t34[:])
                    else:
                        nc.gpsimd.dma_start(zt[:, half], t34[:],
                                            accum_op=mybir.AluOpType.add)

                if b in (0, BPC - 1):
                    # first batch: start the scan before half-1's B-proj
                    # lands; last batch: start the tail chain ASAP.
                    pre_rot(0)
                    scan(0)
                    pre_rot(1)
                else:
                    pre_rot_merged()
                    scan(0)
                post_rot(0)
                scan(1)
                post_rot(1)
                dsb[b] = None

                # ---- u-phase convolution first (no DVE dependency) ----
                oj = [None] * Q
                for j in range(Q):
                    oj[j] = pso.tile([HIDDEN, K], f32, tag="oj",
                                     name=f"oj{b}_{j}")
                    for i in range(j + 1):
                        nc.tensor.matmul(oj[j][:], m2w_t[:, j - i],
                                         inT[b][:, i],
                                         start=(i == 0), stop=False)
                # ---- z-term, half-major so h0 runs during h1's scan ----
                for half in range(2):
                    for j in range(Q):
                        for wt in range(2):
                            last = (half == 1 and wt == 1)
                            nc.tensor.matmul(
                                oj[j][:, 1:K], cw_t[:, half, j, wt],
                                zt[:, half, wt, 0:K - 1],
                                start=False, stop=last)
                for j in range(Q):
                    oT = obuf_pool.tile([HIDDEN, K], fp16, tag="oT")
                    nc.scalar.copy(oT[:], oj[j][:])
                    nc.sync.dma_start(out[b, j], oT[:])

    nc.compile()
    return nc


def _host_constants(A_diag, G_diag, steps, B, C, D):
    A = A_diag.astype(np.float64)
    G = G_diag.astype(np.float64)
    st = steps.astype(np.float64)
    step = 1.0 / (1.0 + np.exp(-st))
    g = np.maximum(G, 0.0)
    denom = np.maximum(step * step, 1e-6)
    s = step * g
    base = np.sqrt(np.maximum(1.0 + s, 1e-6))
    a_low = (2.0 + s - 2.0 * base) / denom
    a_high = (2.0 + s + 2.0 * base) / denom
    a = a_low + np.maximum(A - a_low, 0.0) - np.maximum(A - a_high, 0.0)
    S = 1.0 / (1.0 + step * g)
    T = S + 1.0 - step * step * S * a
    imag = np.sqrt(np.maximum(S - 0.25 * T * T, 0.0))
    lam = 0.5 * T + 1j * imag
    r = np.abs(lam)
    th = np.angle(lam)

    Bc = B[..., 0].astype(np.float64) + 1j * B[..., 1].astype(np.float64)
    Cc = C[..., 0].astype(np.float64) + 1j * C[..., 1].astype(np.float64)

    # bw[h, half, comp, j, p] = {Re,Im}(lam^{3-j} Bc)[p, h]
    bw = np.zeros((HIDDEN, 2, 2, Q, 128), np.float16)
    # cw[p, half, j, wt, h]: wt0 = Re(C lam^{j+1})^T, wt1 = -Im(C lam^{j+1})^T
    cw = np.zeros((128, 2, Q, 2, HIDDEN), np.float16)
    for j in range(Q):
        Wj = (lam ** (Q - 1 - j))[:, None] * Bc          # [P, H]
        Cj = Cc * (lam ** (j + 1))[None, :]              # [H, P]
        for half in range(2):
            psl = slice(128 * half, 128 * (half + 1))
            bw[:, half, 0, j] = Wj.real[psl].T
            bw[:, half, 1, j] = Wj.imag[psl].T
            cw[:, half, j, 0] = Cj.real[:, psl].T
            cw[:, half, j, 1] = -Cj.imag[:, psl].T

    # m2w[h_in, d, h_out] = M_d^T with M_d = Re(C lam^d B) (+diag(D) at d=0)
    m2w = np.zeros((HIDDEN, Q, HIDDEN), np.float16)
    for d in range(Q):
        Md = np.real(Cc @ ((lam ** d)[:, None] * Bc))
        if d == 0:
            Md = Md + np.diag(D.astype(np.float64))
        m2w[:, d] = Md.T

    # rotation tables, phase phi = 4*th*k: tab0 = [cos|-sin], tab1 = [sin|cos]
    kk = np.arange(K, dtype=np.float64)
    cos_m = np.cos(Q * th[:, None] * kk[None, :])
    sin_m = np.sin(Q * th[:, None] * kk[None, :])
    epre = np.zeros((128, 2, 2, 2, K), np.float16)
    for half in range(2):
        psl = slice(128 * half, 128 * (half + 1))
        epre[:, half, 0, 0] = cos_m[psl]
        epre[:, half, 0, 1] = -sin_m[psl]
        epre[:, half, 1, 0] = sin_m[psl]
        epre[:, half, 1, 1] = cos_m[psl]

    rcol = np.zeros((128, 2), np.float32)
    r4 = (r ** Q).astype(np.float64)
    rcol[:, 0] = r4[:128]
    rcol[:, 1] = r4[128:]

    return dict(bw=bw, cw=cw, m2w=m2w, epre=epre, rcol=rcol)


def _prep_xin(core_inputs_f32):
    """[BPC, L, H] f32 -> [BPC, H, Q, K] fp16 with xin[b,h,j,k] = u[b,4k+j,h]."""
    a = core_inputs_f32.reshape(BPC, K, Q, HIDDEN)
    return np.ascontiguousarray(a.transpose(0, 3, 2, 1)).astype(np.float16)


def _make_in_maps(inputs, A_diag, G_diag, steps, B, C, D):
    inputs = np.asarray(inputs, np.float32)
    consts = _host_constants(np.asarray(A_diag), np.asarray(G_diag),
                             np.asarray(steps), np.asarray(B), np.asarray(C),
                             np.asarray(D))
    in_maps = []
    for core in range(N_CORES):
        m = dict(consts)
        m["xin"] = _prep_xin(inputs[BPC * core: BPC * (core + 1)])
        in_maps.append(m)
    return in_maps


def kernel(inputs, A_diag, G_diag, steps, B, C, D):
    from concourse import bass_utils

    in_maps = _make_in_maps(inputs, A_diag, G_diag, steps, B, C, D)
    if "prog" not in _COMPILED:
        _COMPILED["prog"] = _build_program()
    nc = _COMPILED["prog"]

    res = bass_utils.run_bass_kernel_spmd(nc, in_maps,
                                          core_ids=list(range(N_CORES)))
    out = np.concatenate([res.results[i]["out"] for i in range(N_CORES)],
                         axis=0)                      # [B, Q, H, K] fp16
    # out[b, j, h, k] -> full[b, 4k+j, h]
    full = out.astype(np.float32).transpose(0, 3, 1, 2)   # [B, K, Q, H]
    return np.ascontiguousarray(full.reshape(BATCH, LENGTH, HIDDEN))


# revision 16
# speedup vs baseline: 1.0493x; 1.0493x over previous
"""DampedLinOSSLayer Trainium2 kernel v5 (8 NeuronCores, batch-sharded).

Radix-4 time decimation on top of the gauge-factorized scan:
  x_t = lam x_{t-1} + B u_t  decimated by Q=4: z_k = lam^4 z_{k-1} + d_k with
  d_k = sum_j lam^{3-j} B u_{4k+j}.  The lam^{3-j} factors fold into 4 copies
  of the B weights, so the intra-group reduction runs on the PE (4
  accumulating matmuls over phase-deinterleaved input columns).  The DVE only
  rotates/scans the decimated (L/4) grid: gauge y_k = r^4 y_{k-1} + cf_k with
  cf = e^{-i4th k} d; one merged scan instruction covers all four
  (half, comp) segments via zeroed coefficient columns at segment starts.
  Output for t = 4k+j:
    out = Re(C lam^{j+1} z_{k-1}) + sum_{i<=j} M_{j-i} u_{4k+i}
  with M_d = Re(C lam^d B) + diag(D)[d=0].  The z-planes are combined on the
  DVE (z_re, z_im) so the C projection is 2 matmuls per (half, phase); the
  one-column shift of z rides the matmul output AP.  The u-term is a
  10-matmul causal phase convolution.  All rotation tables are plane-views
  of a single e^{+-i4th k} table.

Host side: input is pre-transposed/deinterleaved to [H, 4, 512] per batch
(plain contiguous DMA, no device transpose); output phases [4, H, 512] fp16
are re-interleaved on the host.
"""

import numpy as np

BATCH, LENGTH, HIDDEN, P = 32, 2048, 128, 256
N_CORES = 8
BPC = BATCH // N_CORES
Q = 4
K = LENGTH // Q

XIN_DTYPE = np.float16

_COMPILED = {}


def _build_program():
    import concourse.bacc as bacc
    import concourse.mybir as mybir
    from concourse.tile import TileContext

    f32 = mybir.dt.float32
    fp16 = mybir.dt.float16
    mm = mybir.AluOpType.mult
    ad = mybir.AluOpType.add

    nc = bacc.Bacc("TRN2", target_bir_lowering=False, debug=False,
                   num_devices=N_CORES)

    # host-deinterleaved input: xin[b, h, j, k] = u[b, 4k+j, h]
    xin = nc.dram_tensor("xin", [BPC, HIDDEN, Q, K], fp16,
                         kind="ExternalInput").ap()
    # B weights with lam^{3-j} folded: [h, half, comp, j, p]
    bw = nc.dram_tensor("bw", [HIDDEN, 2, 2, Q, 128], fp16,
                        kind="ExternalInput").ap()
    # per-phase C weights C*lam^{j+1}: [p, half, j, wt(0=re,1=-im), h]
    cw = nc.dram_tensor("cw", [128, 2, Q, 2, HIDDEN], fp16,
                        kind="ExternalInput").ap()
    # phase-convolution weights M_d^T: [h_in, d, h_out]
    m2w = nc.dram_tensor("m2w", [HIDDEN, Q, HIDDEN], fp16,
                         kind="ExternalInput").ap()
    # rotation tables, phase 4*th*k: [p, half, tab, plane, k]
    #   tab0 = [cos | -sin], tab1 = [sin | cos]
    epre = nc.dram_tensor("epre", [128, 2, 2, 2, K], fp16,
                          kind="ExternalInput").ap()
    # r^4 per (p, half) for the scan coefficient
    rcol = nc.dram_tensor("rcol", [128, 2], f32, kind="ExternalInput").ap()
    # output phases; host interleaves
    out = nc.dram_tensor("out", [BPC, Q, HIDDEN, K], fp16,
                         kind="ExternalOutput").ap()

    with TileContext(nc) as tc:
        with (
            tc.tile_pool(name="const", bufs=1) as cpool,
            tc.tile_pool(name="intp", bufs=3) as intr_pool,
            tc.tile_pool(name="dsb", bufs=2) as dsb_pool,
            tc.tile_pool(name="t12", bufs=2) as t12_pool,
            tc.tile_pool(name="cbuf", bufs=2) as cbuf_pool,
            tc.tile_pool(name="ybuf", bufs=2) as ybuf_pool,
            tc.tile_pool(name="xbuf", bufs=2) as xbuf_pool,
            tc.tile_pool(name="obuf", bufs=8) as obuf_pool,
            tc.tile_pool(name="psb", bufs=2, space="PSUM") as psb,
            tc.tile_pool(name="pso", bufs=4, space="PSUM") as pso,
        ):
            bw_t = cpool.tile([HIDDEN, 2, 2, Q, 128], fp16, tag="bw")
            cw_t = cpool.tile([128, 2, Q, 2, HIDDEN], fp16, tag="cw")
            m2w_t = cpool.tile([HIDDEN, Q, HIDDEN], fp16, tag="m2w")
            epre_t = cpool.tile([128, 2, 2, 2, K], fp16, tag="epre")
            rcol_t = cpool.tile([128, 2], f32, tag="rcol")
            # DMA order = need order: batch-0 B-proj wants bw+xin first, the
            # first pre-rotation wants epre+rcol, the C projection cw/m2w.
            inT = [None] * (BPC + 1)
            inT[0] = intr_pool.tile([HIDDEN, Q, K], fp16, tag="inT",
                                    name="inT0")
            nc.sync.dma_start(rcol_t[:], rcol[:])
            nc.sync.dma_start(bw_t[:, 0, 0], bw[:, 0, 0])
            nc.sync.dma_start(inT[0][:], xin[0])
            nc.sync.dma_start(bw_t[:, 0, 1], bw[:, 0, 1])
            nc.sync.dma_start(bw_t[:, 1], bw[:, 1])
            nc.sync.dma_start(epre_t[:, 0], epre[:, 0])
            nc.sync.dma_start(epre_t[:, 1], epre[:, 1])
            nc.sync.dma_start(m2w_t[:], m2w[:])
            nc.sync.dma_start(cw_t[:], cw[:])

            # scan coefficient [128, (half comp k)]: r^4, zero at segment
            # starts so one scan instruction covers 4 independent segments.
            rz_t = cpool.tile([128, 2, 2, K], f32, tag="rz")
            for half in range(2):
                nc.vector.memset(rz_t[:, half], 1.0)
                nc.vector.tensor_scalar_mul(
                    rz_t[:, half], rz_t[:, half],
                    rcol_t[:, half:half + 1])
            nc.vector.memset(rz_t[:, :, :, 0:1], 0.0)

            dsb = [None] * (BPC + 1)

            def bproj(b):
                # B-projection of batch b: 4 accumulating matmuls per
                # (half, comp) over the phase-deinterleaved input columns,
                # then PSUM->SBUF fp16 copies into one [p, half, comp, k]
                # tile.
                ds = dsb_pool.tile([128, 2, 2, K], fp16, tag="ds",
                                   name=f"ds{b}")
                for half in range(2):
                    d = psb.tile([128, 2, K], f32, tag="d")
                    for comp in range(2):
                        for j in range(Q):
                            nc.tensor.matmul(
                                d[:, comp], bw_t[:, half, comp, j],
                                inT[b][:, j], start=(j == 0), stop=(j == Q - 1))
                    nc.scalar.copy(ds[:, half], d[:])
                dsb[b] = ds

            bproj(0)
            for b in range(BPC):
                # prefetch + B-proj one batch ahead so the DVE chain of
                # batch b never waits on the PE/ScalarE front end.
                if b + 1 < BPC:
                    inT[b + 1] = intr_pool.tile([HIDDEN, Q, K], fp16,
                                                tag="inT",
                                                name=f"inT{b + 1}")
                    nc.sync.dma_start(inT[b + 1][:], xin[b + 1])
                    bproj(b + 1)

                # ---- per-half DVE chain; h0 scan/post overlap h1 ----
                cf = cbuf_pool.tile([128, 2, 2, K], fp16, tag="cf",
                                    name=f"cf{b}")
                yb = ybuf_pool.tile([128, 2, 2, K], fp16, tag="y",
                                    name=f"y{b}")
                zt = xbuf_pool.tile([128, 2, 2, K], fp16, tag="zt",
                                    name=f"zt{b}")

                def pre_rot(half):
                    # cf_re = d_re cos + d_im sin ; cf_im = -d_re sin + d_im cos
                    t12 = t12_pool.tile([128, 2, 2, K], fp16, tag="t12",
                                        name=f"t12_{b}_{half}")
                    nc.vector.tensor_mul(
                        t12[:, 0],
                        dsb[b][:, half, 0:1, :].to_broadcast([128, 2, K]),
                        epre_t[:, half, 0])
                    nc.vector.tensor_mul(
                        t12[:, 1],
                        dsb[b][:, half, 1:2, :].to_broadcast([128, 2, K]),
                        epre_t[:, half, 1])
                    nc.vector.tensor_add(cf[:, half], t12[:, 0], t12[:, 1])

                def pre_rot_merged():
                    # both halves in three FD-2048 ops (less fixed overhead)
                    t12 = t12_pool.tile([128, 2, 2, 2, K], fp16, tag="t12",
                                        name=f"t12m_{b}")
                    nc.vector.tensor_mul(
                        t12[:, 0],
                        dsb[b][:, :, 0:1, :].to_broadcast([128, 2, 2, K]),
                        epre_t[:, :, 0])
                    nc.vector.tensor_mul(
                        t12[:, 1],
                        dsb[b][:, :, 1:2, :].to_broadcast([128, 2, 2, K]),
                        epre_t[:, :, 1])
                    nc.vector.tensor_add(cf[:], t12[:, 0], t12[:, 1])

                def scan(half):
                    nc.vector.tensor_tensor_scan(
                        yb[:, half].rearrange("p c k -> p (c k)"),
                        rz_t[:, half].rearrange("p c k -> p (c k)"),
                        cf[:, half].rearrange("p c k -> p (c k)"), 0.0,
                        op0=mm, op1=ad)

                def post_rot(half):
                    # z_re = y_re cos - y_im sin ; z_im = y_re sin + y_im cos
                    # the sum rides a SWDGE SBUF->SBUF DMA (CCE add), keeping
                    # the DVE to the two products.
                    t34 = t12_pool.tile([128, 2, K], fp16, tag="t34",
                                        name=f"t34_{b}_{half}")
                    nc.vector.tensor_mul(
                        zt[:, half],
                        yb[:, half, 0:1, :].to_broadcast([128, 2, K]),
                        epre_t[:, half, :, 0, :])
                    nc.vector.tensor_mul(
                        t34[:],
                        yb[:, half, 1:2, :].to_broadcast([128, 2, K]),
                        epre_t[:, half, :, 1, :])
                    if b == BPC - 1 and half == 1:
                        # tail: the DVE add (~0.7us) beats the SWDGE CCE-add
                        # round trip (~2-3us) when nothing overlaps it.
                        nc.vector.tensor_add(zt[:, half], zt[:, half], t34[:])
                    else:
                        nc.gpsimd.dma_start(zt[:, half], t34[:],
                                            accum_op=mybir.AluOpType.add)

                if b in (0, BPC - 1):
                    # first batch: start the scan before half-1's B-proj
                    # lands; last batch: start the tail chain ASAP.
                    pre_rot(0)
                    scan(0)
                    pre_rot(1)
                else:
                    pre_rot_merged()
                    scan(0)
                post_rot(0)
                scan(1)
                post_rot(1)
                dsb[b] = None

                # ---- u-phase convolution first (no DVE dependency) ----
                oj = [None] * Q
                for j in range(Q):
                    oj[j] = pso.tile([HIDDEN, K], f32, tag="oj",
                                     name=f"oj{b}_{j}")
                    for i in range(j + 1):
                        nc.tensor.matmul(oj[j][:], m2w_t[:, j - i],
                                         inT[b][:, i],
                                         start=(i == 0), stop=False)
                # ---- z-term, half-major so h0 runs during h1's scan ----
                for half in range(2):
                    for j in range(Q):
                        for wt in range(2):
                            last = (half == 1 and wt == 1)
                            nc.tensor.matmul(
                                oj[j][:, 1:K], cw_t[:, half, j, wt],
                                zt[:, half, wt, 0:K - 1],
                                start=False, stop=last)
                for j in range(Q):
                    oT = obuf_pool.tile([HIDDEN, K], fp16, tag="oT")
                    nc.scalar.copy(oT[:], oj[j][:])
                    nc.sync.dma_start(out[b, j], oT[:])

    nc.compile()
    return nc


def _host_constants(A_diag, G_diag, steps, B, C, D):
    A = A_diag.astype(np.float64)
    G = G_diag.astype(np.float64)
    st = steps.astype(np.float64)
    step = 1.0 / (1.0 + np.exp(-st))
    g = np.maximum(G, 0.0)
    denom = np.maximum(step * step, 1e-6)
    s = step * g
    base = np.sqrt(np.maximum(1.0 + s, 1e-6))
    a_low = (2.0 + s - 2.0 * base) / denom
    a_high = (2.0 + s + 2.0 * base) / denom
    a = a_low + np.maximum(A - a_low, 0.0) - np.maximum(A - a_high, 0.0)
    S = 1.0 / (1.0 + step * g)
    T = S + 1.0 - step * step * S * a
    imag = np.sqrt(np.maximum(S - 0.25 * T * T, 0.0))
    lam = 0.5 * T + 1j * imag
    r = np.abs(lam)
    th = np.angle(lam)

    Bc = B[..., 0].astype(np.float64) + 1j * B[..., 1].astype(np.float64)
    Cc = C[..., 0].astype(np.float64) + 1j * C[..., 1].astype(np.float64)

    # bw[h, half, comp, j, p] = {Re,Im}(lam^{3-j} Bc)[p, h]
    bw = np.zeros((HIDDEN, 2, 2, Q, 128), np.float16)
    # cw[p, half, j, wt, h]: wt0 = Re(C lam^{j+1})^T, wt1 = -Im(C lam^{j+1})^T
    cw = np.zeros((128, 2, Q, 2, HIDDEN), np.float16)
    for j in range(Q):
        Wj = (lam ** (Q - 1 - j))[:, None] * Bc          # [P, H]
        Cj = Cc * (lam ** (j + 1))[None, :]              # [H, P]
        for half in range(2):
            psl = slice(128 * half, 128 * (half + 1))
            bw[:, half, 0, j] = Wj.real[psl].T
            bw[:, half, 1, j] = Wj.imag[psl].T
            cw[:, half, j, 0] = Cj.real[:, psl].T
            cw[:, half, j, 1] = -Cj.imag[:, psl].T

    # m2w[h_in, d, h_out] = M_d^T with M_d = Re(C lam^d B) (+diag(D) at d=0)
    m2w = np.zeros((HIDDEN, Q, HIDDEN), np.float16)
    for d in range(Q):
        Md = np.real(Cc @ ((lam ** d)[:, None] * Bc))
        if d == 0:
            Md = Md + np.diag(D.astype(np.float64))
        m2w[:, d] = Md.T

    # rotation tables, phase phi = 4*th*k: tab0 = [cos|-sin], tab1 = [sin|cos]
    kk = np.arange(K, dtype=np.float64)
    cos_m = np.cos(Q * th[:, None] * kk[None, :])
    sin_m = np.sin(Q * th[:, None] * kk[None, :])
    epre = np.zeros((128, 2, 2, 2, K), np.float16)
    for half in range(2):
        psl = slice(128 * half, 128 * (half + 1))
        epre[:, half, 0, 0] = cos_m[psl]
        epre[:, half, 0, 1] = -sin_m[psl]
        epre[:, half, 1, 0] = sin_m[psl]
        epre[:, half, 1, 1] = cos_m[psl]

    rcol = np.zeros((128, 2), np.float32)
    r4 = (r ** Q).astype(np.float64)
    rcol[:, 0] = r4[:128]
    rcol[:, 1] = r4[128:]

    return dict(bw=bw, cw=cw, m2w=m2w, epre=epre, rcol=rcol)


def _prep_xin(core_inputs_f32):
    """[BPC, L, H] f32 -> [BPC, H, Q, K] fp16 with xin[b,h,j,k] = u[b,4k+j,h]."""
    a = core_inputs_f32.reshape(BPC, K, Q, HIDDEN)
    return np.ascontiguousarray(a.transpose(0, 3, 2, 1)).astype(np.float16)


def _make_in_maps(inputs, A_diag, G_diag, steps, B, C, D):
    inputs = np.asarray(inputs, np.float32)
    consts = _host_constants(np.asarray(A_diag), np.asarray(G_diag),
                             np.asarray(steps), np.asarray(B), np.asarray(C),
                             np.asarray(D))
    in_maps = []
    for core in range(N_CORES):
        m = dict(consts)
        m["xin"] = _prep_xin(inputs[BPC * core: BPC * (core + 1)])
        in_maps.append(m)
    return in_maps


def kernel(inputs, A_diag, G_diag, steps, B, C, D):
    from concourse import bass_utils

    in_maps = _make_in_maps(inputs, A_diag, G_diag, steps, B, C, D)
    if "prog" not in _COMPILED:
        _COMPILED["prog"] = _build_program()
    nc = _COMPILED["prog"]

    res = bass_utils.run_bass_kernel_spmd(nc, in_maps,
                                          core_ids=list(range(N_CORES)))
    out = np.concatenate([res.results[i]["out"] for i in range(N_CORES)],
                         axis=0)                      # [B, Q, H, K] fp16
    # out[b, j, h, k] -> full[b, 4k+j, h]
    full = out.astype(np.float32).transpose(0, 3, 1, 2)   # [B, K, Q, H]
    return np.ascontiguousarray(full.reshape(BATCH, LENGTH, HIDDEN))
